# revision 20
# baseline (speedup 1.0000x reference)
"""Distance-aware transformer encoder layer on 8 Trainium2 NeuronCores.

Sharding: core c handles batch b = c//2 and query-half qh = c%2 (512 query
rows). K/V are computed per-core for the full 1024-key sequence of its batch
(duplicated across the core pair). Scores, softmax, out-proj, LayerNorms and
the FFN are perfectly sharded by query rows. No collectives.

v2 redesign vs baseline (648us):
- No tiny-descriptor DMAs: identity/ones built on-chip, per-partition bias
  columns loaded as a [48,128] block + one PE transpose, [128,D] broadcasts
  done with PE ones-matmuls. Loads split across the two HWDGE queues
  (sync: srcT + weights in consumption order; scalar: distT/rows/src_q).
- Distance bias folded multiplicatively: E = exp(scores) * (d+eps)^(-|s|),
  with expb precomputed once (bf16) and fused per-tile by one DVE multiply.
  Removes the 128 identity bias-matmuls from the PE stream.
- bf16 srcT and bf16 weights (halves HBM traffic); kt/qt/scores stay fp32r.
- Softmax denominators: ones-column trick + reciprocal_approx_fast + PE
  ones-matmul broadcast (no DRAM bounce).
- 14-instruction PE warm-up on on-chip data (p-state ramp) instead of 160
  DMA-dependent spins.
"""

import numpy as np
import ml_dtypes

import bass_rust
import concourse.bass as bass
import concourse.tile as tile
import concourse.mybir as mybir
from concourse.bass_utils import run_bass_kernel_spmd
from concourse.masks import make_identity

B, S, D, H, DFF, HD = 4, 1024, 1024, 16, 4096, 64
SQ = 512          # query rows per core
NCORES = 8
EPS = 1e-5
F32 = mybir.dt.float32
F32R = mybir.dt.float32r
BF16 = mybir.dt.bfloat16
FT = mybir.ActivationFunctionType
ALU = mybir.AluOpType

_nop_ctr = [0]


def _legalize_waits(nc):
    """walrus codegen in this toolchain accepts only one sync-wait per
    instruction; split extras onto same-engine NoOps inserted before."""
    n_fixed = 0
    for f in nc.m.functions:
        for bb in f.blocks:
            insts = bb.instructions
            i = 0
            while i < len(insts):
                inst = insts[i]
                si = inst.sync_info
                waits = list(si.on_wait) if si is not None and si.on_wait else []
                if len(waits) > 1:
                    keep = waits[-1]
                    for w in waits[:-1]:
                        n = bass_rust.InstNoOp(
                            name=f"waitsplit-nop-{_nop_ctr[0]}", ins=[], outs=[]
                        )
                        _nop_ctr[0] += 1
                        n.engine = inst.engine
                        n.sync_info = bass_rust.SyncInfo(on_update=[], on_wait=[w])
                        insts.insert(i, n)
                        i += 1
                    inst.sync_info = bass_rust.SyncInfo(
                        on_update=list(si.on_update or []), on_wait=[keep]
                    )
                    n_fixed += 1
                i += 1
    return n_fixed


def _build():
    nc = bass.Bass()
    dp = nc.declare_dram_parameter

    SrcT = dp("srcT", [D, S], BF16, isOutput=False)        # src[b][perm].T
    SrcQ = dp("src_q", [SQ, D], F32, isOutput=False)       # query rows (+bo)
    ExpB = dp("expb", [S, SQ], BF16, isOutput=False)       # (dist^T+eps)^(-|s|)
    Wk = dp("Wk", [D, D], BF16, isOutput=False)
    Wq = dp("Wq", [D, D], BF16, isOutput=False)            # pre-scaled HD^-0.5
    Wv = dp("Wv", [D, D], BF16, isOutput=False)
    Wo = dp("Wo", [D, D], BF16, isOutput=False)
    W1 = dp("W1", [D, DFF], BF16, isOutput=False)
    W2 = dp("W2", [DFF, D], BF16, isOutput=False)
    Bias2D = dp("bias2d", [48, 128], F32R, isOutput=False)  # bq_s|bk|b1 chunks
    BvR = dp("bv_r", [1, D], F32R, isOutput=False)
    B2R = dp("b2_r", [1, D], F32R, isOutput=False)
    GRows = dp("g_rows", [4, D], F32R, isOutput=False)     # g1|be1|g2|be2
    Out = dp("out", [SQ, D], F32, isOutput=True)

    with tile.TileContext(nc) as tc:
        import contextlib

        ctx = contextlib.ExitStack()
        with ctx:
            consts = ctx.enter_context(tc.tile_pool(name="consts", bufs=1))
            asrc = ctx.enter_context(tc.tile_pool(name="asrc", bufs=1))
            a4b = ctx.enter_context(tc.tile_pool(name="a4b", bufs=1))
            a2 = ctx.enter_context(tc.tile_pool(name="a2", bufs=1))
            persist = ctx.enter_context(tc.tile_pool(name="persist", bufs=1))
            wpool = ctx.enter_context(tc.tile_pool(name="wpool", bufs=3))
            epool = ctx.enter_context(tc.tile_pool(name="epool", bufs=3))
            lnbig = ctx.enter_context(tc.tile_pool(name="lnbig", bufs=1))
            small = ctx.enter_context(tc.tile_pool(name="small", bufs=2))
            ps_s = ctx.enter_context(tc.tile_pool(name="ps_s", bufs=2, space="PSUM"))
            ps_mm = ctx.enter_context(tc.tile_pool(name="ps_mm", bufs=4, space="PSUM"))

            # ---- on-chip constants (no DMA) ----
            stage = consts.tile([128, 128], F32, tag="stage")
            make_identity(nc, stage)
            ident = consts.tile([128, 128], F32R, tag="ident")
            nc.vector.tensor_copy(out=ident, in_=stage)
            onesrow = consts.tile([1, 128], F32R, tag="onesrow")
            nc.vector.memset(stage[0:1, 0:128], 1.0)
            nc.vector.tensor_copy(out=onesrow, in_=stage[0:1, 0:128])
            ones2 = consts.tile([128, 128], F32R, tag="ones2")
            nc.vector.memset(stage, 1.0)
            nc.vector.tensor_copy(out=ones2, in_=stage)
            log_eps = consts.tile([128, 1], F32, tag="log_eps")
            nc.vector.memset(log_eps, 1e-9)
            ln_eps = consts.tile([128, 1], F32, tag="ln_eps")
            nc.vector.memset(ln_eps, EPS)

            # ---- critical loads first, split across both HWDGE queues ----
            srcT = asrc.tile([128, 8, 1024], BF16, tag="asrc")
            for co in range(4):
                nc.sync.dma_start(
                    out=srcT[:, co, :], in_=SrcT[co * 128 : co * 128 + 128, :]
                )
            for co in range(4, 8):
                nc.scalar.dma_start(
                    out=srcT[:, co, :], in_=SrcT[co * 128 : co * 128 + 128, :]
                )
            wk = wpool.tile([128, 8, 1024], BF16, tag="wbig")
            nc.scalar.dma_start(
                out=wk, in_=Wk.rearrange("(ko ki) m -> ki ko m", ki=128)
            )
            bias2d = consts.tile([48, 128], F32R, tag="bias2d")
            nc.scalar.dma_start(out=bias2d, in_=Bias2D[:, :])
            bv_sb = consts.tile([1, D], F32R, tag="bv_sb")
            nc.scalar.dma_start(out=bv_sb, in_=BvR[:, :])
            b2_sb = consts.tile([1, D], F32R, tag="b2_sb")
            nc.scalar.dma_start(out=b2_sb, in_=B2R[:, :])

            # ---- PE warm-up spin (p-state ramp), no DMA dependence ----
            spinps = ps_mm.tile([128, 512], F32, tag="psmm")
            for _ in range(30):
                nc.tensor.matmul(spinps[:, 0:128], ident, ident, start=True, stop=True)

            # ---- per-partition bias columns via one PE transpose ----
            # cbias[:, 0:8]=bq_s, [:, 8:16]=bk, [:, 16:48]=b1
            psB = ps_mm.tile([128, 512], F32R, tag="psmm")
            nc.tensor.transpose(psB[:, 0:48], bias2d[0:48, :], ident[0:48, 0:48])
            cbias = consts.tile([128, 48], F32, tag="cbias")
            nc.vector.tensor_copy(out=cbias, in_=psB[:, 0:48])

            # ---- expb[k, ko, q] = (dist^T + 1e-9)^(-|s|), host-precomputed ----
            expb = persist.tile([128, 8, 512], BF16, tag="expb")
            for ko in range(8):
                nc.scalar.dma_start(
                    out=expb[:, ko, :], in_=ExpB[ko * 128 : ko * 128 + 128, :]
                )

            # ---- K^T projection: kt[dout, keys], fp32r ----
            kt = a4b.tile([128, 16, 1024], BF16, tag="a4b")
            for dt in range(8):
                for nt in range(2):
                    psum = ps_mm.tile([128, 512], F32, tag="psmm")
                    for ko in range(8):
                        nc.tensor.matmul(
                            psum,
                            wk[:, ko, dt * 128 : dt * 128 + 128],
                            srcT[:, ko, nt * 512 : nt * 512 + 512],
                            start=(ko == 0),
                            stop=(ko == 7),
                        )
                    nc.vector.tensor_scalar_add(
                        out=kt[:, dt, nt * 512 : nt * 512 + 512],
                        in0=psum,
                        scalar1=cbias[:, 8 + dt : 9 + dt],
                    )

            wq = wpool.tile([128, 8, 1024], BF16, tag="wbig")
            nc.sync.dma_start(
                out=wq, in_=Wq.rearrange("(ko ki) m -> ki ko m", ki=128)
            )

            # ---- Q^T projection: qt[dout, q], fp32r ----
            qt = a2.tile([128, 8, 1024], BF16, tag="a2")
            for dt in range(8):
                psum = ps_mm.tile([128, 512], F32, tag="psmm")
                for ko in range(8):
                    nc.tensor.matmul(
                        psum,
                        wq[:, ko, dt * 128 : dt * 128 + 128],
                        srcT[:, ko, 0:512],
                        start=(ko == 0),
                        stop=(ko == 7),
                    )
                nc.vector.tensor_scalar_add(
                    out=qt[:, dt, 0:512], in0=psum, scalar1=cbias[:, dt : dt + 1]
                )

            wv = wpool.tile([128, 8, 1024], BF16, tag="wbig")
            nc.sync.dma_start(
                out=wv, in_=Wv.rearrange("(ko ki) m -> ki ko m", ki=128)
            )

            # ---- V projection (keys on partitions + ones column), bf16 ----
            v_sb = persist.tile([128, 8, 16, 65], BF16, tag="v_sb")
            for mt in range(8):
                nc.gpsimd.memset(v_sb[:, mt, :, 64:65], 1.0)
            for vc2 in range(2):
                for mt in range(8):
                    psum = ps_mm.tile([128, 512], F32, tag="psmm")
                    nc.tensor.matmul(
                        psum,
                        onesrow[0:1, 0:128],
                        bv_sb[0:1, vc2 * 512 : vc2 * 512 + 512],
                        start=True,
                        stop=False,
                    )
                    for ko in range(8):
                        nc.tensor.matmul(
                            psum,
                            srcT[:, ko, mt * 128 : mt * 128 + 128],
                            wv[:, ko, vc2 * 512 : vc2 * 512 + 512],
                            start=False,
                            stop=(ko == 7),
                        )
                    nc.vector.tensor_copy(
                        out=v_sb[:, mt, vc2 * 8 : vc2 * 8 + 8, 0:64],
                        in_=psum.rearrange("p (h e) -> p h e", e=64),
                    )

            wo = wpool.tile([128, 8, 1024], BF16, tag="wbig")
            nc.sync.dma_start(
                out=wo, in_=Wo.rearrange("(dp ki) d -> ki dp d", ki=128)
            )

            # ---- g/beta broadcast to [128, D] bf16 via PE ones-matmuls ----
            g1b = consts.tile([128, D], BF16, tag="g1b")
            be1b = consts.tile([128, D], BF16, tag="be1b")
            g2b = consts.tile([128, D], BF16, tag="g2b")
            be2b = consts.tile([128, D], BF16, tag="be2b")
            for r, dst in enumerate((g1b, be1b, g2b, be2b)):
                grow = consts.tile([1, D], F32R, tag="grow")
                nc.scalar.dma_start(out=grow, in_=GRows[r : r + 1, :])
                for half in range(2):
                    psG = ps_mm.tile([128, 512], F32, tag="psmm")
                    nc.tensor.matmul(
                        psG,
                        onesrow[0:1, 0:128],
                        grow[0:1, half * 512 : half * 512 + 512],
                        start=True,
                        stop=True,
                    )
                    nc.vector.tensor_copy(
                        out=dst[:, half * 512 : half * 512 + 512], in_=psG
                    )

            # src_q on the scalar queue, dispatched before the attention exps
            src_q = persist.tile([128, 4, 1024], F32, tag="src_q")
            for qt_i in range(4):
                nc.scalar.dma_start(
                    out=src_q[:, qt_i, :],
                    in_=SrcQ[qt_i * 128 : qt_i * 128 + 128, :],
                )

            # ---- attention, head by head (unnormalized; denominators saved) ----
            ao_sb = persist.tile([128, 8, 512], BF16, tag="ao_sb")
            steps = [(h, kog) for h in range(H) for kog in range(4)]
            pao = {}
            pend = []          # (e_t, h, kog) awaiting attnV, depth 2
            for idx in range(len(steps) + 2):
                if idx < len(steps):
                    h, kog = steps[idx]
                    base = (h % 2) * 64
                    dt = h // 2
                    if kog == 0:
                        pao[h] = ps_mm.tile([128, 512], F32, tag="psmm", name=f"pao{h}")
                    pss = ps_s.tile([128, 2, 512], F32, tag="pss")
                    for kl in range(2):
                        ko = kog * 2 + kl
                        nc.tensor.matmul(
                            pss[:, kl, :],
                            kt[base : base + 64, dt, ko * 128 : ko * 128 + 128],
                            qt[base : base + 64, dt, 0:512],
                            start=True,
                            stop=True,
                        )
                    e_t = epool.tile([128, 2, 512], BF16, tag="e_t")
                    nc.scalar.activation(out=e_t, in_=pss, func=FT.Exp)
                    # split the bias multiply across DVE and GPSIMD to halve
                    # per-engine SBUF traffic (both operands are SBUF)
                    eng = nc.vector if idx % 2 == 0 else nc.gpsimd
                    eng.tensor_mul(
                        out=e_t, in0=e_t, in1=expb[:, kog * 2 : kog * 2 + 2, :]
                    )
                    pend.append((e_t, h, kog))
                if len(pend) > 2 or (idx >= len(steps) and pend):
                    pe_t, ph, pkog = pend.pop(0)
                    for kl in range(2):
                        ko = pkog * 2 + kl
                        nc.tensor.matmul(
                            pao[ph][0:65, :],
                            v_sb[:, ko, ph, :],
                            pe_t[:, kl, :],
                            start=(ko == 0),
                            stop=(ko == 7),
                        )
                    if pkog == 3:
                        pbase = (ph % 2) * 64
                        ppao = pao.pop(ph)
                        nc.scalar.activation(
                            out=ppao[64:65, :], in_=ppao[64:65, :], func=FT.Ln
                        )
                        recs = small.tile([128, 512], F32R, tag="recs")
                        nc.scalar.activation(
                            out=recs[64:65, :],
                            in_=ppao[64:65, :],
                            func=FT.Exp,
                            scale=-1.0,
                        )
                        psR = ps_mm.tile([128, 512], F32, tag="psmm")
                        nc.tensor.matmul(
                            psR[0:64, :],
                            ones2[64:65, 0:64],
                            recs[64:65, :],
                            start=True,
                            stop=True,
                        )
                        R_sb = small.tile([64, 512], BF16, tag="R_sb")
                        nc.vector.tensor_copy(out=R_sb, in_=psR[0:64, :])
                        nc.vector.tensor_mul(
                            out=ao_sb[pbase : pbase + 64, ph // 2, :],
                            in0=ppao[0:64, :],
                            in1=R_sb,
                        )

            # ---- out projection + residual; x = LN1(src + ao@Wo + bo) ----
            x_sb = asrc.tile([128, 4, 1024], F32R, tag="asrc")
            xT = persist.tile([128, 8, 512], BF16, tag="expb")  # reuse expb
            xpre_all = a2.tile([128, 4, 1024], F32, tag="a2")
            for nt in range(2):
                for qt_i in range(4):
                    psum = ps_mm.tile([128, 512], F32, tag="psmm")
                    for dpi in range(8):
                        nc.tensor.matmul(
                            psum,
                            ao_sb[:, dpi, qt_i * 128 : qt_i * 128 + 128],
                            wo[:, dpi, nt * 512 : nt * 512 + 512],
                            start=(dpi == 0),
                            stop=(dpi == 7),
                        )
                    nc.vector.tensor_add(
                        out=xpre_all[:, qt_i, nt * 512 : nt * 512 + 512],
                        in0=psum,
                        in1=src_q[:, qt_i, nt * 512 : nt * 512 + 512],
                    )
            for qt_i in range(4):
                xpre = xpre_all[:, qt_i, :]
                stats = small.tile([128, 2, 6], F32, tag="stats")
                for half in range(2):
                    nc.vector.bn_stats(
                        out=stats[:, half, :],
                        in_=xpre[:, half * 512 : half * 512 + 512],
                    )
                mv = small.tile([128, 2], F32, tag="mv")
                nc.vector.bn_aggr(out=mv, in_=stats)
                sq = small.tile([128, 1], F32, tag="sq")
                nc.scalar.activation(out=sq, in_=mv[:, 1:2], func=FT.Sqrt, bias=ln_eps)
                rstd = small.tile([128, 1], F32, tag="rstd")
                nc.vector.reciprocal(out=rstd, in_=sq)
                nmr = small.tile([128, 1], F32, tag="nmr")
                nc.vector.tensor_scalar(
                    out=nmr,
                    in0=mv[:, 0:1],
                    scalar1=rstd,
                    scalar2=-1.0,
                    op0=ALU.mult,
                    op1=ALU.mult,
                )
                xn = lnbig.tile([128, 1024], F32, tag="xn")
                nc.scalar.activation(
                    out=xn, in_=xpre, func=FT.Identity, bias=nmr, scale=rstd
                )
                xg = lnbig.tile([128, 1024], F32, tag="xg")
                nc.vector.tensor_mul(out=xg, in0=xn, in1=g1b)
                nc.vector.tensor_add(out=x_sb[:, qt_i, :], in0=xg, in1=be1b)
                for ct in range(8):
                    pt = ps_mm.tile([128, 512], F32R, tag="psmm")
                    nc.tensor.transpose(
                        pt[:, 0:128],
                        x_sb[:, qt_i, ct * 128 : ct * 128 + 128],
                        ident,
                    )
                    nc.vector.tensor_copy(
                        out=xT[:, ct, qt_i * 128 : qt_i * 128 + 128],
                        in_=pt[:, 0:128],
                    )

            # ---- FFN mm1 + relu: h[f, q] bf16 ----
            h_sb = a4b.tile([128, 32, 512], BF16, tag="a4b")
            for fc in range(4):
                w1c = wpool.tile([128, 8, 1024], BF16, tag="wbig")
                nc.sync.dma_start(
                    out=w1c,
                    in_=W1[:, fc * 1024 : fc * 1024 + 1024].rearrange(
                        "(co ki) f -> ki co f", ki=128
                    ),
                )
                for fl in range(8):
                    ft = fc * 8 + fl
                    psum = ps_mm.tile([128, 512], F32, tag="psmm")
                    for co in range(8):
                        nc.tensor.matmul(
                            psum,
                            w1c[:, co, fl * 128 : fl * 128 + 128],
                            xT[:, co, :],
                            start=(co == 0),
                            stop=(co == 7),
                        )
                    nc.scalar.activation(
                        out=h_sb[:, ft, :],
                        in_=psum,
                        func=FT.Relu,
                        bias=cbias[:, 16 + ft : 17 + ft],
                    )

            # ---- FFN mm2 + residual; out = LN2(x + h@W2 + b2) ----
            ypre_all = a2.tile([128, 4, 1024], F32, tag="a2")
            for nt in range(2):
                w2a = wpool.tile([128, 16, 512], BF16, tag="wbig")
                nc.sync.dma_start(
                    out=w2a,
                    in_=W2[0:2048, nt * 512 : nt * 512 + 512].rearrange(
                        "(fo fi) d -> fi fo d", fi=128
                    ),
                )
                w2b = wpool.tile([128, 16, 512], BF16, tag="wbig")
                nc.sync.dma_start(
                    out=w2b,
                    in_=W2[2048:4096, nt * 512 : nt * 512 + 512].rearrange(
                        "(fo fi) d -> fi fo d", fi=128
                    ),
                )
                for qt_i in range(4):
                    psum = ps_mm.tile([128, 512], F32, tag="psmm")
                    nc.tensor.matmul(
                        psum,
                        onesrow[0:1, 0:128],
                        b2_sb[0:1, nt * 512 : nt * 512 + 512],
                        start=True,
                        stop=False,
                    )
                    for ft in range(16):
                        nc.tensor.matmul(
                            psum,
                            h_sb[:, ft, qt_i * 128 : qt_i * 128 + 128],
                            w2a[:, ft, :],
                            start=False,
                            stop=False,
                        )
                    for ft in range(16):
                        nc.tensor.matmul(
                            psum,
                            h_sb[:, 16 + ft, qt_i * 128 : qt_i * 128 + 128],
                            w2b[:, ft, :],
                            start=False,
                            stop=(ft == 15),
                        )
                    nc.vector.tensor_add(
                        out=ypre_all[:, qt_i, nt * 512 : nt * 512 + 512],
                        in0=psum,
                        in1=x_sb[:, qt_i, nt * 512 : nt * 512 + 512],
                    )

            for qt_i in range(4):
                ypre = ypre_all[:, qt_i, :]
                stats = small.tile([128, 2, 6], F32, tag="stats")
                for half in range(2):
                    nc.vector.bn_stats(
                        out=stats[:, half, :],
                        in_=ypre[:, half * 512 : half * 512 + 512],
                    )
                mv = small.tile([128, 2], F32, tag="mv")
                nc.vector.bn_aggr(out=mv, in_=stats)
                sq = small.tile([128, 1], F32, tag="sq")
                nc.scalar.activation(out=sq, in_=mv[:, 1:2], func=FT.Sqrt, bias=ln_eps)
                rstd = small.tile([128, 1], F32, tag="rstd")
                nc.vector.reciprocal(out=rstd, in_=sq)
                nmr = small.tile([128, 1], F32, tag="nmr")
                nc.vector.tensor_scalar(
                    out=nmr,
                    in0=mv[:, 0:1],
                    scalar1=rstd,
                    scalar2=-1.0,
                    op0=ALU.mult,
                    op1=ALU.mult,
                )
                yn = lnbig.tile([128, 1024], F32, tag="xn")
                nc.scalar.activation(
                    out=yn, in_=ypre, func=FT.Identity, bias=nmr, scale=rstd
                )
                yg = lnbig.tile([128, 1024], F32, tag="xg")
                nc.vector.tensor_mul(out=yg, in0=yn, in1=g2b)
                out_t = lnbig.tile([128, 1024], F32, tag="xn")
                nc.vector.tensor_add(out=out_t, in0=yg, in1=be2b)
                nc.sync.dma_start(
                    out=Out[qt_i * 128 : qt_i * 128 + 128, :], in_=out_t
                )

    _legalize_waits(nc)
    return nc


_CACHE = {}


def kernel(**inputs):
    import os

    if "nc" not in _CACHE:
        _CACHE["nc"] = _build()
    nc = _CACHE["nc"]

    f32 = np.float32
    bf16 = ml_dtypes.bfloat16
    src = np.asarray(inputs["src"], f32)
    distances = np.asarray(inputs["distances"], f32)
    scale = np.float32(HD ** -0.5)
    Wq_s = (np.asarray(inputs["Wq"], f32) * scale).astype(bf16)
    bq_s = (np.asarray(inputs["bq"], f32) * scale).astype(f32)
    negabs = -abs(float(np.asarray(inputs["dist_scale"])))

    bias2d = np.concatenate(
        [
            bq_s.reshape(8, 128),
            np.asarray(inputs["bk"], f32).reshape(8, 128),
            np.asarray(inputs["b1"], f32).reshape(32, 128),
        ],
        axis=0,
    )  # [48, 128]

    shared = {
        "Wq": Wq_s,
        "Wk": np.asarray(inputs["Wk"], f32).astype(bf16),
        "Wv": np.asarray(inputs["Wv"], f32).astype(bf16),
        "Wo": np.asarray(inputs["Wo"], f32).astype(bf16),
        "W1": np.asarray(inputs["W1"], f32).astype(bf16),
        "W2": np.asarray(inputs["W2"], f32).astype(bf16),
        "bias2d": np.ascontiguousarray(bias2d),
        "bv_r": np.asarray(inputs["bv"], f32).reshape(1, D).copy(),
        "b2_r": np.asarray(inputs["b2"], f32).reshape(1, D).copy(),
        "g_rows": np.stack(
            [
                np.asarray(inputs["g1"], f32),
                np.asarray(inputs["beta1"], f32),
                np.asarray(inputs["g2"], f32),
                np.asarray(inputs["beta2"], f32),
            ]
        ),
    }

    bo = np.asarray(inputs["bo"], f32)
    in_maps = []
    for c in range(NCORES):
        b, qh = c // 2, c % 2
        q0 = qh * SQ
        if qh == 0:
            perm = np.arange(S)
        else:
            perm = np.r_[np.arange(512, 1024), np.arange(0, 512)]
        m = dict(shared)
        m["srcT"] = np.ascontiguousarray(src[b][perm].T).astype(bf16)
        m["src_q"] = np.ascontiguousarray(src[b, q0 : q0 + SQ] + bo[None, :])
        dT = distances[b, q0 : q0 + SQ][:, perm].T + np.float32(1e-9)
        m["expb"] = np.ascontiguousarray(np.power(dT, negabs, dtype=np.float64)).astype(bf16)
        in_maps.append(m)

    trace = bool(int(os.environ.get("BASS_KERNEL_TRACE", "0")))
    res = run_bass_kernel_spmd(
        nc,
        in_maps,
        core_ids=list(range(NCORES)),
        trace=trace,
        stitch_traces=False,
    )
    _CACHE["last_result"] = res

    out = np.empty((B, S, D), f32)
    for c in range(NCORES):
        b, qh = c // 2, c % 2
        out[b, qh * SQ : qh * SQ + SQ] = res.results[c]["out"]
    return out


# revision 22
# speedup vs baseline: 1.0522x; 1.0522x over previous
"""Distance-aware transformer encoder layer on 8 Trainium2 NeuronCores.

Sharding: core c handles batch b = c//2 and query-half qh = c%2 (512 query
rows). K/V are computed per-core for the full 1024-key sequence of its batch
(duplicated across the core pair). Scores, softmax, out-proj, LayerNorms and
the FFN are perfectly sharded by query rows. No collectives.

v2 redesign vs baseline (648us):
- No tiny-descriptor DMAs: identity/ones built on-chip, per-partition bias
  columns loaded as a [48,128] block + one PE transpose, [128,D] broadcasts
  done with PE ones-matmuls. Loads split across the two HWDGE queues
  (sync: srcT + weights in consumption order; scalar: distT/rows/src_q).
- Distance bias folded multiplicatively: E = exp(scores) * (d+eps)^(-|s|),
  with expb precomputed once (bf16) and fused per-tile by one DVE multiply.
  Removes the 128 identity bias-matmuls from the PE stream.
- bf16 srcT and bf16 weights (halves HBM traffic); kt/qt/scores stay fp32r.
- Softmax denominators: ones-column trick + reciprocal_approx_fast + PE
  ones-matmul broadcast (no DRAM bounce).
- 14-instruction PE warm-up on on-chip data (p-state ramp) instead of 160
  DMA-dependent spins.
"""

import numpy as np
import ml_dtypes

import bass_rust
import concourse.bass as bass
import concourse.tile as tile
import concourse.mybir as mybir
from concourse.bass_utils import run_bass_kernel_spmd
from concourse.masks import make_identity

B, S, D, H, DFF, HD = 4, 1024, 1024, 16, 4096, 64
SQ = 512          # query rows per core
NCORES = 8
EPS = 1e-5
F32 = mybir.dt.float32
F32R = mybir.dt.float32r
BF16 = mybir.dt.bfloat16
FT = mybir.ActivationFunctionType
ALU = mybir.AluOpType

_nop_ctr = [0]


def _legalize_waits(nc):
    """walrus codegen in this toolchain accepts only one sync-wait per
    instruction; split extras onto same-engine NoOps inserted before."""
    n_fixed = 0
    for f in nc.m.functions:
        for bb in f.blocks:
            insts = bb.instructions
            i = 0
            while i < len(insts):
                inst = insts[i]
                si = inst.sync_info
                waits = list(si.on_wait) if si is not None and si.on_wait else []
                if len(waits) > 1:
                    keep = waits[-1]
                    for w in waits[:-1]:
                        n = bass_rust.InstNoOp(
                            name=f"waitsplit-nop-{_nop_ctr[0]}", ins=[], outs=[]
                        )
                        _nop_ctr[0] += 1
                        n.engine = inst.engine
                        n.sync_info = bass_rust.SyncInfo(on_update=[], on_wait=[w])
                        insts.insert(i, n)
                        i += 1
                    inst.sync_info = bass_rust.SyncInfo(
                        on_update=list(si.on_update or []), on_wait=[keep]
                    )
                    n_fixed += 1
                i += 1
    return n_fixed


def _build():
    nc = bass.Bass()
    dp = nc.declare_dram_parameter

    SrcT = dp("srcT", [D, S], BF16, isOutput=False)        # src[b][perm].T
    SrcQ = dp("src_q", [SQ, D], F32, isOutput=False)       # query rows (+bo)
    ExpB = dp("expb", [S, SQ], BF16, isOutput=False)       # (dist^T+eps)^(-|s|)
    Wk = dp("Wk", [D, D], BF16, isOutput=False)
    Wq = dp("Wq", [D, D], BF16, isOutput=False)            # pre-scaled HD^-0.5
    Wv = dp("Wv", [D, D], BF16, isOutput=False)
    Wo = dp("Wo", [D, D], BF16, isOutput=False)
    W1 = dp("W1", [D, DFF], BF16, isOutput=False)
    W2 = dp("W2", [DFF, D], BF16, isOutput=False)
    Bias2D = dp("bias2d", [48, 128], F32R, isOutput=False)  # bq_s|bk|b1 chunks
    BvR = dp("bv_r", [1, D], F32R, isOutput=False)
    B2R = dp("b2_r", [1, D], F32R, isOutput=False)
    GRows = dp("g_rows", [4, D], F32R, isOutput=False)     # g1|be1|g2|be2
    Out = dp("out", [SQ, D], F32, isOutput=True)

    with tile.TileContext(nc) as tc:
        import contextlib

        ctx = contextlib.ExitStack()
        with ctx:
            consts = ctx.enter_context(tc.tile_pool(name="consts", bufs=1))
            asrc = ctx.enter_context(tc.tile_pool(name="asrc", bufs=1))
            a4b = ctx.enter_context(tc.tile_pool(name="a4b", bufs=1))
            a2 = ctx.enter_context(tc.tile_pool(name="a2", bufs=1))
            persist = ctx.enter_context(tc.tile_pool(name="persist", bufs=1))
            wpool = ctx.enter_context(tc.tile_pool(name="wpool", bufs=3))
            epool = ctx.enter_context(tc.tile_pool(name="epool", bufs=3))
            lnbig = ctx.enter_context(tc.tile_pool(name="lnbig", bufs=1))
            small = ctx.enter_context(tc.tile_pool(name="small", bufs=2))
            ps_s = ctx.enter_context(tc.tile_pool(name="ps_s", bufs=2, space="PSUM"))
            ps_mm = ctx.enter_context(tc.tile_pool(name="ps_mm", bufs=4, space="PSUM"))

            # ---- on-chip constants (no DMA) ----
            stage = consts.tile([128, 128], F32, tag="stage")
            make_identity(nc, stage)
            ident = consts.tile([128, 128], F32R, tag="ident")
            nc.vector.tensor_copy(out=ident, in_=stage)
            onesrow = consts.tile([1, 128], F32R, tag="onesrow")
            nc.vector.memset(stage[0:1, 0:128], 1.0)
            nc.vector.tensor_copy(out=onesrow, in_=stage[0:1, 0:128])
            ones2 = consts.tile([128, 128], F32R, tag="ones2")
            nc.vector.memset(stage, 1.0)
            nc.vector.tensor_copy(out=ones2, in_=stage)
            log_eps = consts.tile([128, 1], F32, tag="log_eps")
            nc.vector.memset(log_eps, 1e-9)
            ln_eps = consts.tile([128, 1], F32, tag="ln_eps")
            nc.vector.memset(ln_eps, EPS)

            # ---- critical loads first, split across both HWDGE queues ----
            srcT = asrc.tile([128, 8, 1024], BF16, tag="asrc")
            for co in range(4):
                nc.sync.dma_start(
                    out=srcT[:, co, :], in_=SrcT[co * 128 : co * 128 + 128, :]
                )
            for co in range(4, 8):
                nc.scalar.dma_start(
                    out=srcT[:, co, :], in_=SrcT[co * 128 : co * 128 + 128, :]
                )
            wk = wpool.tile([128, 8, 1024], BF16, tag="wbig")
            nc.scalar.dma_start(
                out=wk, in_=Wk.rearrange("(ko ki) m -> ki ko m", ki=128)
            )
            bias2d = consts.tile([48, 128], F32R, tag="bias2d")
            nc.scalar.dma_start(out=bias2d, in_=Bias2D[:, :])
            bv_sb = consts.tile([1, D], F32R, tag="bv_sb")
            nc.scalar.dma_start(out=bv_sb, in_=BvR[:, :])
            b2_sb = consts.tile([1, D], F32R, tag="b2_sb")
            nc.scalar.dma_start(out=b2_sb, in_=B2R[:, :])

            # ---- PE warm-up spin (p-state ramp), no DMA dependence ----
            spinps = ps_mm.tile([128, 512], F32, tag="psmm")
            for _ in range(30):
                nc.tensor.matmul(spinps[:, 0:128], ident, ident, start=True, stop=True)

            # ---- per-partition bias columns via one PE transpose ----
            # cbias[:, 0:8]=bq_s, [:, 8:16]=bk, [:, 16:48]=b1
            psB = ps_mm.tile([128, 512], F32R, tag="psmm")
            nc.tensor.transpose(psB[:, 0:48], bias2d[0:48, :], ident[0:48, 0:48])
            cbias = consts.tile([128, 48], F32, tag="cbias")
            nc.vector.tensor_copy(out=cbias, in_=psB[:, 0:48])

            # ---- expb[k, ko, q] = (dist^T + 1e-9)^(-|s|), host-precomputed ----
            expb = persist.tile([128, 8, 512], BF16, tag="expb")
            for ko in range(8):
                nc.scalar.dma_start(
                    out=expb[:, ko, :], in_=ExpB[ko * 128 : ko * 128 + 128, :]
                )

            # ---- K^T projection: kt[dout, keys], fp32r ----
            kt = a4b.tile([128, 16, 1024], BF16, tag="a4b")
            for dt in range(8):
                for nt in range(2):
                    psum = ps_mm.tile([128, 512], F32, tag="psmm")
                    for ko in range(8):
                        nc.tensor.matmul(
                            psum,
                            wk[:, ko, dt * 128 : dt * 128 + 128],
                            srcT[:, ko, nt * 512 : nt * 512 + 512],
                            start=(ko == 0),
                            stop=(ko == 7),
                        )
                    nc.vector.tensor_scalar_add(
                        out=kt[:, dt, nt * 512 : nt * 512 + 512],
                        in0=psum,
                        scalar1=cbias[:, 8 + dt : 9 + dt],
                    )

            wq = wpool.tile([128, 8, 1024], BF16, tag="wbig")
            nc.sync.dma_start(
                out=wq, in_=Wq.rearrange("(ko ki) m -> ki ko m", ki=128)
            )

            # ---- Q^T projection: qt[dout, q], fp32r ----
            qt = a2.tile([128, 8, 1024], BF16, tag="a2")
            for dt in range(8):
                psum = ps_mm.tile([128, 512], F32, tag="psmm")
                for ko in range(8):
                    nc.tensor.matmul(
                        psum,
                        wq[:, ko, dt * 128 : dt * 128 + 128],
                        srcT[:, ko, 0:512],
                        start=(ko == 0),
                        stop=(ko == 7),
                    )
                nc.vector.tensor_scalar_add(
                    out=qt[:, dt, 0:512], in0=psum, scalar1=cbias[:, dt : dt + 1]
                )

            wv = wpool.tile([128, 8, 1024], BF16, tag="wbig")
            nc.sync.dma_start(
                out=wv, in_=Wv.rearrange("(ko ki) m -> ki ko m", ki=128)
            )

            # ---- V projection (keys on partitions + ones column), bf16 ----
            v_sb = persist.tile([128, 8, 16, 65], BF16, tag="v_sb")
            for mt in range(8):
                nc.gpsimd.memset(v_sb[:, mt, :, 64:65], 1.0)
            for vc2 in range(2):
                for mt in range(8):
                    psum = ps_mm.tile([128, 512], F32, tag="psmm")
                    nc.tensor.matmul(
                        psum,
                        onesrow[0:1, 0:128],
                        bv_sb[0:1, vc2 * 512 : vc2 * 512 + 512],
                        start=True,
                        stop=False,
                    )
                    for ko in range(8):
                        nc.tensor.matmul(
                            psum,
                            srcT[:, ko, mt * 128 : mt * 128 + 128],
                            wv[:, ko, vc2 * 512 : vc2 * 512 + 512],
                            start=False,
                            stop=(ko == 7),
                        )
                    nc.vector.tensor_copy(
                        out=v_sb[:, mt, vc2 * 8 : vc2 * 8 + 8, 0:64],
                        in_=psum.rearrange("p (h e) -> p h e", e=64),
                    )

            wo = wpool.tile([128, 8, 1024], BF16, tag="wbig")
            nc.sync.dma_start(
                out=wo, in_=Wo.rearrange("(dp ki) d -> ki dp d", ki=128)
            )

            # ---- g/beta broadcast to [128, D] bf16 via PE ones-matmuls ----
            g1b = consts.tile([128, D], BF16, tag="g1b")
            be1b = consts.tile([128, D], BF16, tag="be1b")
            g2b = consts.tile([128, D], BF16, tag="g2b")
            be2b = consts.tile([128, D], BF16, tag="be2b")
            for r, dst in enumerate((g1b, be1b, g2b, be2b)):
                grow = consts.tile([1, D], F32R, tag="grow")
                nc.scalar.dma_start(out=grow, in_=GRows[r : r + 1, :])
                for half in range(2):
                    psG = ps_mm.tile([128, 512], F32, tag="psmm")
                    nc.tensor.matmul(
                        psG,
                        onesrow[0:1, 0:128],
                        grow[0:1, half * 512 : half * 512 + 512],
                        start=True,
                        stop=True,
                    )
                    nc.vector.tensor_copy(
                        out=dst[:, half * 512 : half * 512 + 512], in_=psG
                    )

            # src_q on the scalar queue, dispatched before the attention exps
            src_q = persist.tile([128, 4, 1024], F32, tag="src_q")
            for qt_i in range(4):
                nc.scalar.dma_start(
                    out=src_q[:, qt_i, :],
                    in_=SrcQ[qt_i * 128 : qt_i * 128 + 128, :],
                )

            # ---- attention, head by head (unnormalized; denominators saved) ----
            ao_sb = persist.tile([128, 8, 512], BF16, tag="ao_sb")
            for h in range(H):
                base = (h % 2) * 64
                dt = h // 2
                pao = ps_mm.tile([128, 512], F32, tag="psmm")
                for kog in range(4):
                    pss = ps_s.tile([128, 2, 512], F32, tag="pss")
                    for kl in range(2):
                        ko = kog * 2 + kl
                        nc.tensor.matmul(
                            pss[:, kl, :],
                            kt[base : base + 64, dt, ko * 128 : ko * 128 + 128],
                            qt[base : base + 64, dt, 0:512],
                            start=True,
                            stop=True,
                        )
                    e_t = epool.tile([128, 2, 512], BF16, tag="e_t")
                    nc.scalar.activation(out=e_t, in_=pss, func=FT.Exp)
                    nc.vector.tensor_mul(
                        out=e_t, in0=e_t, in1=expb[:, kog * 2 : kog * 2 + 2, :]
                    )
                    for kl in range(2):
                        ko = kog * 2 + kl
                        nc.tensor.matmul(
                            pao[0:65, :],
                            v_sb[:, ko, h, :],
                            e_t[:, kl, :],
                            start=(ko == 0),
                            stop=(ko == 7),
                        )
                # 1/den = exp(-ln(den)) on the scalar engine, in place at
                # partition 64 (single-partition ops must stay same-base)
                nc.scalar.activation(
                    out=pao[64:65, :], in_=pao[64:65, :], func=FT.Ln
                )
                recs = small.tile([128, 512], F32R, tag="recs")
                nc.scalar.activation(
                    out=recs[64:65, :], in_=pao[64:65, :], func=FT.Exp, scale=-1.0
                )
                psR = ps_mm.tile([128, 512], F32, tag="psmm")
                nc.tensor.matmul(
                    psR[0:64, :],
                    ones2[64:65, 0:64],
                    recs[64:65, :],
                    start=True,
                    stop=True,
                )
                R_sb = small.tile([64, 512], BF16, tag="R_sb")
                nc.vector.tensor_copy(out=R_sb, in_=psR[0:64, :])
                nc.vector.tensor_mul(
                    out=ao_sb[base : base + 64, dt, :],
                    in0=pao[0:64, :],
                    in1=R_sb,
                )

            # ---- out projection + residual; x = LN1(src + ao@Wo + bo) ----
            x_sb = asrc.tile([128, 4, 1024], F32R, tag="asrc")
            xT = persist.tile([128, 8, 512], BF16, tag="expb")  # reuse expb
            xpre_all = a2.tile([128, 4, 1024], F32, tag="a2")
            for nt in range(2):
                for qt_i in range(4):
                    psum = ps_mm.tile([128, 512], F32, tag="psmm")
                    for dpi in range(8):
                        nc.tensor.matmul(
                            psum,
                            ao_sb[:, dpi, qt_i * 128 : qt_i * 128 + 128],
                            wo[:, dpi, nt * 512 : nt * 512 + 512],
                            start=(dpi == 0),
                            stop=(dpi == 7),
                        )
                    nc.vector.tensor_add(
                        out=xpre_all[:, qt_i, nt * 512 : nt * 512 + 512],
                        in0=psum,
                        in1=src_q[:, qt_i, nt * 512 : nt * 512 + 512],
                    )
            for qt_i in range(4):
                xpre = xpre_all[:, qt_i, :]
                stats = small.tile([128, 2, 6], F32, tag="stats")
                for half in range(2):
                    nc.vector.bn_stats(
                        out=stats[:, half, :],
                        in_=xpre[:, half * 512 : half * 512 + 512],
                    )
                mv = small.tile([128, 2], F32, tag="mv")
                nc.vector.bn_aggr(out=mv, in_=stats)
                sq = small.tile([128, 1], F32, tag="sq")
                nc.scalar.activation(out=sq, in_=mv[:, 1:2], func=FT.Sqrt, bias=ln_eps)
                rstd = small.tile([128, 1], F32, tag="rstd")
                nc.vector.reciprocal(out=rstd, in_=sq)
                nmr = small.tile([128, 1], F32, tag="nmr")
                nc.vector.tensor_scalar(
                    out=nmr,
                    in0=mv[:, 0:1],
                    scalar1=rstd,
                    scalar2=-1.0,
                    op0=ALU.mult,
                    op1=ALU.mult,
                )
                xn = lnbig.tile([128, 1024], F32, tag="xn")
                nc.scalar.activation(
                    out=xn, in_=xpre, func=FT.Identity, bias=nmr, scale=rstd
                )
                xg = lnbig.tile([128, 1024], F32, tag="xg")
                nc.vector.tensor_mul(out=xg, in0=xn, in1=g1b)
                nc.vector.tensor_add(out=x_sb[:, qt_i, :], in0=xg, in1=be1b)
                for ct in range(8):
                    pt = ps_mm.tile([128, 512], F32R, tag="psmm")
                    nc.tensor.transpose(
                        pt[:, 0:128],
                        x_sb[:, qt_i, ct * 128 : ct * 128 + 128],
                        ident,
                    )
                    nc.scalar.activation(
                        out=xT[:, ct, qt_i * 128 : qt_i * 128 + 128],
                        in_=pt[:, 0:128],
                        func=FT.Copy,
                    )

            # ---- FFN mm1 + relu: h[f, q] bf16 ----
            h_sb = a4b.tile([128, 32, 512], BF16, tag="a4b")
            for fc in range(4):
                w1c = wpool.tile([128, 8, 1024], BF16, tag="wbig")
                nc.sync.dma_start(
                    out=w1c,
                    in_=W1[:, fc * 1024 : fc * 1024 + 1024].rearrange(
                        "(co ki) f -> ki co f", ki=128
                    ),
                )
                for fl in range(8):
                    ft = fc * 8 + fl
                    psum = ps_mm.tile([128, 512], F32, tag="psmm")
                    for co in range(8):
                        nc.tensor.matmul(
                            psum,
                            w1c[:, co, fl * 128 : fl * 128 + 128],
                            xT[:, co, :],
                            start=(co == 0),
                            stop=(co == 7),
                        )
                    nc.scalar.activation(
                        out=h_sb[:, ft, :],
                        in_=psum,
                        func=FT.Relu,
                        bias=cbias[:, 16 + ft : 17 + ft],
                    )

            # ---- FFN mm2 + residual; out = LN2(x + h@W2 + b2) ----
            ypre_all = a2.tile([128, 4, 1024], F32, tag="a2")
            for nt in range(2):
                w2a = wpool.tile([128, 16, 512], BF16, tag="wbig")
                nc.sync.dma_start(
                    out=w2a,
                    in_=W2[0:2048, nt * 512 : nt * 512 + 512].rearrange(
                        "(fo fi) d -> fi fo d", fi=128
                    ),
                )
                w2b = wpool.tile([128, 16, 512], BF16, tag="wbig")
                nc.sync.dma_start(
                    out=w2b,
                    in_=W2[2048:4096, nt * 512 : nt * 512 + 512].rearrange(
                        "(fo fi) d -> fi fo d", fi=128
                    ),
                )
                for qt_i in range(4):
                    psum = ps_mm.tile([128, 512], F32, tag="psmm")
                    nc.tensor.matmul(
                        psum,
                        onesrow[0:1, 0:128],
                        b2_sb[0:1, nt * 512 : nt * 512 + 512],
                        start=True,
                        stop=False,
                    )
                    for ft in range(16):
                        nc.tensor.matmul(
                            psum,
                            h_sb[:, ft, qt_i * 128 : qt_i * 128 + 128],
                            w2a[:, ft, :],
                            start=False,
                            stop=False,
                        )
                    for ft in range(16):
                        nc.tensor.matmul(
                            psum,
                            h_sb[:, 16 + ft, qt_i * 128 : qt_i * 128 + 128],
                            w2b[:, ft, :],
                            start=False,
                            stop=(ft == 15),
                        )
                    nc.vector.tensor_add(
                        out=ypre_all[:, qt_i, nt * 512 : nt * 512 + 512],
                        in0=psum,
                        in1=x_sb[:, qt_i, nt * 512 : nt * 512 + 512],
                    )

            for qt_i in range(4):
                ypre = ypre_all[:, qt_i, :]
                stats = small.tile([128, 2, 6], F32, tag="stats")
                for half in range(2):
                    nc.vector.bn_stats(
                        out=stats[:, half, :],
                        in_=ypre[:, half * 512 : half * 512 + 512],
                    )
                mv = small.tile([128, 2], F32, tag="mv")
                nc.vector.bn_aggr(out=mv, in_=stats)
                sq = small.tile([128, 1], F32, tag="sq")
                nc.scalar.activation(out=sq, in_=mv[:, 1:2], func=FT.Sqrt, bias=ln_eps)
                rstd = small.tile([128, 1], F32, tag="rstd")
                nc.vector.reciprocal(out=rstd, in_=sq)
                nmr = small.tile([128, 1], F32, tag="nmr")
                nc.vector.tensor_scalar(
                    out=nmr,
                    in0=mv[:, 0:1],
                    scalar1=rstd,
                    scalar2=-1.0,
                    op0=ALU.mult,
                    op1=ALU.mult,
                )
                yn = lnbig.tile([128, 1024], F32, tag="xn")
                nc.scalar.activation(
                    out=yn, in_=ypre, func=FT.Identity, bias=nmr, scale=rstd
                )
                yg = lnbig.tile([128, 1024], F32, tag="xg")
                nc.vector.tensor_mul(out=yg, in0=yn, in1=g2b)
                out_t = lnbig.tile([128, 1024], F32, tag="xn")
                nc.vector.tensor_add(out=out_t, in0=yg, in1=be2b)
                nc.sync.dma_start(
                    out=Out[qt_i * 128 : qt_i * 128 + 128, :], in_=out_t
                )

    _legalize_waits(nc)
    return nc


_CACHE = {}


def kernel(**inputs):
    import os

    if "nc" not in _CACHE:
        _CACHE["nc"] = _build()
    nc = _CACHE["nc"]

    f32 = np.float32
    bf16 = ml_dtypes.bfloat16
    src = np.asarray(inputs["src"], f32)
    distances = np.asarray(inputs["distances"], f32)
    scale = np.float32(HD ** -0.5)
    Wq_s = (np.asarray(inputs["Wq"], f32) * scale).astype(bf16)
    bq_s = (np.asarray(inputs["bq"], f32) * scale).astype(f32)
    negabs = -abs(float(np.asarray(inputs["dist_scale"])))

    bias2d = np.concatenate(
        [
            bq_s.reshape(8, 128),
            np.asarray(inputs["bk"], f32).reshape(8, 128),
            np.asarray(inputs["b1"], f32).reshape(32, 128),
        ],
        axis=0,
    )  # [48, 128]

    shared = {
        "Wq": Wq_s,
        "Wk": np.asarray(inputs["Wk"], f32).astype(bf16),
        "Wv": np.asarray(inputs["Wv"], f32).astype(bf16),
        "Wo": np.asarray(inputs["Wo"], f32).astype(bf16),
        "W1": np.asarray(inputs["W1"], f32).astype(bf16),
        "W2": np.asarray(inputs["W2"], f32).astype(bf16),
        "bias2d": np.ascontiguousarray(bias2d),
        "bv_r": np.asarray(inputs["bv"], f32).reshape(1, D).copy(),
        "b2_r": np.asarray(inputs["b2"], f32).reshape(1, D).copy(),
        "g_rows": np.stack(
            [
                np.asarray(inputs["g1"], f32),
                np.asarray(inputs["beta1"], f32),
                np.asarray(inputs["g2"], f32),
                np.asarray(inputs["beta2"], f32),
            ]
        ),
    }

    bo = np.asarray(inputs["bo"], f32)
    in_maps = []
    for c in range(NCORES):
        b, qh = c // 2, c % 2
        q0 = qh * SQ
        if qh == 0:
            perm = np.arange(S)
        else:
            perm = np.r_[np.arange(512, 1024), np.arange(0, 512)]
        m = dict(shared)
        m["srcT"] = np.ascontiguousarray(src[b][perm].T).astype(bf16)
        m["src_q"] = np.ascontiguousarray(src[b, q0 : q0 + SQ] + bo[None, :])
        dT = distances[b, q0 : q0 + SQ][:, perm].T + np.float32(1e-9)
        m["expb"] = np.ascontiguousarray(np.power(dT, negabs, dtype=np.float64)).astype(bf16)
        in_maps.append(m)

    trace = bool(int(os.environ.get("BASS_KERNEL_TRACE", "0")))
    res = run_bass_kernel_spmd(
        nc,
        in_maps,
        core_ids=list(range(NCORES)),
        trace=trace,
        stitch_traces=False,
    )
    _CACHE["last_result"] = res

    out = np.empty((B, S, D), f32)
    for c in range(NCORES):
        b, qh = c // 2, c % 2
        out[b, qh * SQ : qh * SQ + SQ] = res.results[c]["out"]
    return out
